# revision 11
# baseline (speedup 1.0000x reference)
"""GCN link predictor on 8 trn2 NeuronCores (Bass/Tile SPMD) — v3.

v1 (13.76 ms) issued ~11.8k per-tile indirect DMAs at ~1.16 us fixed SWDGE
cost each.  v3 replaces every gather with batched dma_gather (custom Q7
ucode): one op moves up to 1024 rows, ops rotate across 4 SWDGE queues so
descriptor generation runs on all 8 Q7 cores.

dma_gather needs int16 indices, so node tables are windowed: 4 windows of
N/4 (<=25000) rows; edge slots are sorted so each gather op touches one
window.  agg slots: [dst-block][window] with per-(b,w) runs padded to 128;
decode slots: 16 (src-window, dst-window) classes, each padded to 128.

bf16 everywhere on the gather path (hhat / AllGather / hf / msg / S / z3);
layer 3 is zero-padded to 128 cols so every gathered row is 256 B.
PSUM accumulation stays fp32.  Scores come back per-slot; the host
un-permutes slots back to edge order.

Self-contained: hardcodes all shapes for the nn_GCNLinkPredictor problem.
"""
import numpy as np

import concourse.bacc as bacc
import concourse.bass as bass
import concourse.mybir as mybir
import concourse.tile as tile
from concourse.bass_utils import run_bass_kernel_spmd

P = 128
N = 100000
E = 1600000
M = 8
NW = 4                       # index windows (N/NW must be <= 32767)
NPC = N // M                 # 12500
BPC = (NPC + P - 1) // P     # 98
SLICE = BPC * P              # 12544
CIN = 128
CH = 128                     # layer width; layer 3 zero-padded to CH
COUT = 64
EPC = E // M                 # 200000
SCT = 32                     # decode superchunk tiles
OPT = 8                      # max tiles per dma_gather op (1024 idxs)

f32 = mybir.dt.float32
bf16 = mybir.dt.bfloat16
i16 = mybir.dt.int16
i32 = mybir.dt.int32


def _configure(n, e):
    global N, E, NPC, BPC, SLICE, EPC
    N, E = n, e
    NPC = N // M
    BPC = (NPC + P - 1) // P
    SLICE = BPC * P
    EPC = E // M


def _w0():
    w0 = N // NW
    assert N % NW == 0 and w0 <= 32767
    return w0


def _pack_idx16(flat):
    """flat slot-ordered local indices -> [128, len/16] int16 (16-part
    wrap, replicated for the 8 Q7 cores)."""
    assert len(flat) % 16 == 0
    a = flat.reshape(-1, 16).T.astype(np.int16)
    return np.ascontiguousarray(np.tile(a, (8, 1)))


# --------------------------- host preprocessing ---------------------------

def _prep_agg(pos_edge_index):
    """Slot layout: for b in blocks: for w in windows: K[b][w] tiles.
    Returns per-core srcidx16 [128, T*8], dstlocT [P, T], dinvT, plus the
    static plan (K table)."""
    w0 = _w0()
    src = np.concatenate([pos_edge_index[0], np.arange(N, dtype=np.int64)])
    dst = np.concatenate([pos_edge_index[1], np.arange(N, dtype=np.int64)])
    deg = np.bincount(dst, minlength=N).astype(np.float32)
    dinv = np.where(deg > 0, 1.0 / np.sqrt(deg), 0.0).astype(np.float32)

    core_of = dst // NPC
    per_core = []
    counts = np.zeros((M, BPC, NW), dtype=np.int64)
    for c in range(M):
        sel = core_of == c
        s_c, d_c = src[sel], dst[sel]
        b_c = (d_c - c * NPC) // P
        w_c = s_c // w0
        order = np.lexsort((w_c, b_c))
        s_c, d_c, b_c, w_c = s_c[order], d_c[order], b_c[order], w_c[order]
        np.add.at(counts[c], (b_c, w_c), 1)
        per_core.append((s_c, d_c, b_c, w_c))

    K = np.max((counts + P - 1) // P, axis=0)        # [BPC, NW] tiles
    KB = K.sum(axis=1)                               # tiles per block
    T = int(KB.sum())

    cores = []
    for c in range(M):
        s_c, d_c, b_c, w_c = per_core[c]
        srcloc = np.zeros(T * P, dtype=np.int16)
        dstloc = np.full(T * P, -1.0, dtype=np.float32)
        # starts of each (b, w) run in the sorted edge list
        bw = b_c * NW + w_c
        starts = np.searchsorted(bw, np.arange(BPC * NW))
        ends = np.searchsorted(bw, np.arange(BPC * NW) + 1)
        tile_off = 0
        for b in range(BPC):
            for w in range(NW):
                n_bw = ends[b * NW + w] - starts[b * NW + w]
                base = tile_off * P
                sl = slice(starts[b * NW + w], ends[b * NW + w])
                srcloc[base:base + n_bw] = (s_c[sl] - w * w0).astype(np.int16)
                dstloc[base:base + n_bw] = (
                    d_c[sl] - (c * NPC + b * P)).astype(np.float32)
                tile_off += K[b, w]
        dinvT = np.zeros((P, BPC), dtype=np.float32)
        for b in range(BPC):
            lo = c * NPC + b * P
            hi = min(lo + P, (c + 1) * NPC)
            dinvT[:hi - lo, b] = dinv[lo:hi]
        cores.append(dict(
            srcidx=_pack_idx16(srcloc),
            dstlocT=np.ascontiguousarray(
                dstloc.reshape(T, P).T),
            dinvT=dinvT))
    return dict(K=K, KB=KB, T=T), cores


def _prep_decode(edge_index):
    """Sort each core's edges into 16 (ws, wd) classes, pad classes to
    128-multiples.  Returns plan (KD per class, TDp) and per-core packed
    s/d idx16 tables plus the slot->edge map."""
    w0 = _w0()
    per_core = []
    counts = np.zeros((M, NW * NW), dtype=np.int64)
    for c in range(M):
        s = edge_index[0, c * EPC:(c + 1) * EPC].astype(np.int64)
        d = edge_index[1, c * EPC:(c + 1) * EPC].astype(np.int64)
        cls = (s // w0) * NW + (d // w0)
        order = np.argsort(cls, kind="stable")
        s, d, cls = s[order], d[order], cls[order]
        np.add.at(counts[c], cls, 1)
        per_core.append((s, d, cls, order))

    KD = np.max((counts + P - 1) // P, axis=0)       # [16] tiles per class
    TDp = int(KD.sum())

    cores = []
    for c in range(M):
        s, d, cls, order = per_core[c]
        sidx = np.zeros(TDp * P, dtype=np.int16)
        didx = np.zeros(TDp * P, dtype=np.int16)
        emap = np.full(TDp * P, -1, dtype=np.int64)
        starts = np.searchsorted(cls, np.arange(NW * NW))
        ends = np.searchsorted(cls, np.arange(NW * NW) + 1)
        toff = 0
        for k in range(NW * NW):
            ws, wd = k // NW, k % NW
            n_k = ends[k] - starts[k]
            base = toff * P
            sl = slice(starts[k], ends[k])
            sidx[base:base + n_k] = (s[sl] - ws * w0).astype(np.int16)
            didx[base:base + n_k] = (d[sl] - wd * w0).astype(np.int16)
            emap[base:base + n_k] = order[sl]
            toff += KD[k]
        cores.append(dict(sidx=_pack_idx16(sidx), didx=_pack_idx16(didx),
                          emap=emap))
    return dict(KD=KD, TDp=TDp), cores


def _gather_ops_agg(plan):
    """Static agg gather op list: (tile_start_global, ntiles, window)."""
    K = plan["K"]
    ops = []
    toff = 0
    for b in range(BPC):
        for w in range(NW):
            k = int(K[b, w])
            o = 0
            while o < k:
                n = min(OPT, k - o)
                ops.append((toff + o, n, w))
                o += n
            toff += k
    return ops


def _gather_ops_decode(plan, side):
    """Static decode gather ops: (tile_start, ntiles, window), broken at
    class boundaries (side window changes) and superchunk boundaries."""
    KD = plan["KD"]
    ops = []
    toff = 0
    for k in range(NW * NW):
        w = (k // NW) if side == 0 else (k % NW)
        kt = int(KD[k])
        o = 0
        while o < kt:
            t = toff + o
            # break at superchunk boundary and OPT cap
            room_sc = SCT - (t % SCT)
            n = min(OPT, kt - o, room_sc)
            ops.append((t, n, w))
            o += n
        toff += kt
    return ops


# ----------------------------- device builder -----------------------------

def build_nc(aplan, dplan_p, dplan_n, reps=1):
    T = aplan["T"]
    K = aplan["K"]
    TDp, TDn = dplan_p["TDp"], dplan_n["TDp"]
    w0 = _w0()

    nc = bacc.Bacc(None, target_bir_lowering=False, num_swdge_queues=4)
    qrot = [0]

    def nextq():
        q = qrot[0]
        qrot[0] = (q + 1) % 4
        return q

    with tile.TileContext(nc) as tc:
        with tc.tile_pool(name="dram", bufs=1, space="DRAM") as dram, \
             tc.tile_pool(name="cst", bufs=1) as cst, \
             tc.tile_pool(name="xt", bufs=3) as xtp, \
             tc.tile_pool(name="sS", bufs=3) as sSp, \
             tc.tile_pool(name="msg", bufs=3) as msgp, \
             tc.tile_pool(name="zb", bufs=4) as zbp, \
             tc.tile_pool(name="dg", bufs=2) as dgp, \
             tc.tile_pool(name="prd", bufs=2) as prdp, \
             tc.tile_pool(name="dix", bufs=3) as dixp, \
             tc.tile_pool(name="ps", bufs=2, space="PSUM") as psp, \
             tc.tile_pool(name="acc", bufs=2, space="PSUM") as accp:

            def ein(name, shape, dtype=f32):
                return dram.tile(shape, dtype, kind="ExternalInput",
                                 name=name, uniquify=False)

            x_s = ein("x_s", [SLICE, CIN])
            ident_in = ein("ident_in", [P, P])
            iota_in = ein("iota_in", [P, P])
            W1 = ein("W1", [CIN, CH]); W2 = ein("W2", [CH, CH])
            W3 = ein("W3", [CH, CH])                      # zero-padded
            bb1 = ein("bb1", [P, CH]); bb2 = ein("bb2", [P, CH])
            bb3 = ein("bb3", [P, CH])                     # zero-padded
            dinvT = ein("dinvT", [P, BPC])
            srcidx = ein("srcidx", [P, T * 8], i16)
            dstlocT = ein("dstlocT", [P, T])
            ps_idx = ein("ps_idx", [P, TDp * 8], i16)
            pd_idx = ein("pd_idx", [P, TDp * 8], i16)
            ns_idx = ein("ns_idx", [P, TDn * 8], i16)
            nd_idx = ein("nd_idx", [P, TDn * 8], i16)

            pos_out = dram.tile([TDp, P], f32, kind="ExternalOutput",
                                name="pos_out", uniquify=False)
            neg_out = dram.tile([TDn, P], f32, kind="ExternalOutput",
                                name="neg_out", uniquify=False)

            hs1 = dram.tile([SLICE, CH], bf16, name="hs1")
            hs2 = dram.tile([SLICE, CH], bf16, name="hs2")
            hs3 = dram.tile([SLICE, CH], bf16, name="hs3")
            z1s = dram.tile([SLICE, CH], f32, name="z1s")
            z2s = dram.tile([SLICE, CH], f32, name="z2s")
            z3s = dram.tile([SLICE, CH], bf16, name="z3s")
            # Shared (collective-output) tensors allow exactly one writer
            # instruction, so with Python-unrolled reps each rep gets its own.
            hf1s = [dram.tile([N, CH], bf16, name=f"hf1_r{r}",
                              addr_space="Shared") for r in range(reps)]
            hf2s = [dram.tile([N, CH], bf16, name=f"hf2_r{r}",
                              addr_space="Shared") for r in range(reps)]
            hf3s = [dram.tile([N, CH], bf16, name=f"hf3_r{r}",
                              addr_space="Shared") for r in range(reps)]
            z3fs = [dram.tile([N, CH], bf16, name=f"z3f_r{r}",
                              addr_space="Shared") for r in range(reps)]

            # ---------------- constants to SBUF ----------------
            W1_sb = cst.tile([CIN, CH], f32)
            W2_sb = cst.tile([CH, CH], f32)
            W3_sb = cst.tile([CH, CH], f32)
            bb1_sb = cst.tile([P, CH], f32)
            bb2_sb = cst.tile([P, CH], f32)
            bb3_sb = cst.tile([P, CH], f32)
            dinv_sb = cst.tile([P, BPC], f32)
            srcidx_sb = cst.tile([P, T * 8], i16)
            dstloc_sb = cst.tile([P, T], f32)
            ident = cst.tile([P, P], f32)
            iota_f = cst.tile([P, P], f32)
            for dst_t, src_t in [(W1_sb, W1), (W2_sb, W2), (W3_sb, W3),
                                 (bb1_sb, bb1), (bb2_sb, bb2), (bb3_sb, bb3),
                                 (dinv_sb, dinvT), (srcidx_sb, srcidx),
                                 (dstloc_sb, dstlocT), (ident, ident_in),
                                 (iota_f, iota_in)]:
                nc.sync.dma_start(out=dst_t[:], in_=src_t[:])

            agg_ops = _gather_ops_agg(aplan)
            dec_ops = {(0, "p"): _gather_ops_decode(dplan_p, 0),
                       (1, "p"): _gather_ops_decode(dplan_p, 1),
                       (0, "n"): _gather_ops_decode(dplan_n, 0),
                       (1, "n"): _gather_ops_decode(dplan_n, 1)}

            # ---------------- phases ----------------
            def dense(z_in, W_sb, hs_out, scope):
                with nc.named_scope(scope):
                    for i in range(BPC):
                        zt = xtp.tile([P, CH], f32, tag="zt")
                        nc.sync.dma_start(out=zt[:],
                                          in_=z_in[i * P:(i + 1) * P, :])
                        tp = psp.tile([P, CH], f32, tag="tp")
                        nc.tensor.transpose(out=tp[:], in_=zt[:],
                                            identity=ident[:])
                        zT = xtp.tile([P, CH], f32, tag="zT")
                        nc.vector.tensor_copy(out=zT[:], in_=tp[:])
                        hp = psp.tile([P, CH], f32, tag="hp")
                        nc.tensor.matmul(out=hp[:], lhsT=zT[:], rhs=W_sb[:],
                                         start=True, stop=True)
                        hh = zbp.tile([P, CH], bf16, tag="hh")
                        nc.vector.tensor_scalar(
                            out=hh[:], in0=hp[:],
                            scalar1=dinv_sb[:, i:i + 1], scalar2=None,
                            op0=mybir.AluOpType.mult)
                        nc.sync.dma_start(
                            out=hs_out[i * P:(i + 1) * P, :], in_=hh[:])

            def allgather(slice_t, full_t, scope):
                with nc.named_scope(scope):
                    nc.gpsimd.collective_compute(
                        "AllGather", mybir.AluOpType.bypass,
                        replica_groups=[list(range(M))],
                        ins=[slice_t[:NPC, :]],
                        outs=[full_t[:]])

            def agg(hf, bias_sb, relu, z_out, zdt, scope):
                with nc.named_scope(scope):
                    opi = 0
                    toff = 0
                    for b in range(BPC):
                        kb = int(K[b].sum())
                        msgw = msgp.tile([P, kb * CH], bf16, tag="msg")
                        m3 = msgw[:].rearrange("p (t c) -> p t c", c=CH)
                        # gathers for this block (per window, <=OPT tiles)
                        while opi < len(agg_ops) and \
                                agg_ops[opi][0] < toff + kb:
                            t0, nt, w = agg_ops[opi]
                            nc.gpsimd.dma_gather(
                                m3[:, t0 - toff:t0 - toff + nt, :],
                                hf[w * w0:(w + 1) * w0, :],
                                srcidx_sb[:, t0 * 8:t0 * 8 + nt * 8],
                                nt * P, nt * P, CH, queue_num=nextq())
                            opi += 1
                        S = sSp.tile([P, kb * P], bf16, tag="S")
                        nc.vector.tensor_tensor(
                            out=S[:].rearrange("p (k q) -> p k q", q=P),
                            in0=dstloc_sb[:, toff:toff + kb][:, :, None]
                                .broadcast_to([P, kb, P]),
                            in1=iota_f[:][:, None, :]
                                .broadcast_to([P, kb, P]),
                            op=mybir.AluOpType.is_equal)
                        acc = accp.tile([P, CH], f32, tag="acc")
                        for k in range(kb):
                            nc.tensor.matmul(
                                out=acc[:],
                                lhsT=S[:, k * P:(k + 1) * P],
                                rhs=msgw[:, k * CH:(k + 1) * CH],
                                start=(k == 0), stop=(k == kb - 1))
                        zb = zbp.tile([P, CH], f32, tag="zb")
                        nc.vector.tensor_scalar(
                            out=zb[:], in0=acc[:],
                            scalar1=dinv_sb[:, b:b + 1], scalar2=None,
                            op0=mybir.AluOpType.mult)
                        if relu:
                            nc.vector.tensor_tensor(
                                out=zb[:], in0=zb[:], in1=bias_sb[:],
                                op=mybir.AluOpType.add)
                            zo = zbp.tile([P, CH], zdt, tag="zo")
                            nc.vector.tensor_scalar_max(zo[:], zb[:], 0.0)
                        else:
                            zo = zbp.tile([P, CH], zdt, tag="zo")
                            nc.vector.tensor_tensor(
                                out=zo[:], in0=zb[:], in1=bias_sb[:],
                                op=mybir.AluOpType.add)
                        nc.sync.dma_start(
                            out=z_out[b * P:(b + 1) * P, :], in_=zo[:])
                        toff += kb

            def decode(plan, z3f, sidx_t, didx_t, ops_s, ops_d, out_t,
                       scope):
                TDx = plan["TDp"]
                nsc = (TDx + SCT - 1) // SCT
                # pre-split op lists by superchunk
                by_sc_s = [[] for _ in range(nsc)]
                for op in ops_s:
                    by_sc_s[op[0] // SCT].append(op)
                by_sc_d = [[] for _ in range(nsc)]
                for op in ops_d:
                    by_sc_d[op[0] // SCT].append(op)
                with nc.named_scope(scope):
                    for sc in range(nsc):
                        nt_sc = min(SCT, TDx - sc * SCT)
                        six = dixp.tile([P, SCT * 8], i16, tag="six")
                        nc.sync.dma_start(
                            out=six[:, :nt_sc * 8],
                            in_=sidx_t[:, sc * SCT * 8:
                                       sc * SCT * 8 + nt_sc * 8])
                        dix = dixp.tile([P, SCT * 8], i16, tag="dix")
                        nc.sync.dma_start(
                            out=dix[:, :nt_sc * 8],
                            in_=didx_t[:, sc * SCT * 8:
                                       sc * SCT * 8 + nt_sc * 8])
                        za = dgp.tile([P, SCT * CH], bf16, tag="za")
                        zd = dgp.tile([P, SCT * CH], bf16, tag="zd")
                        za3 = za[:].rearrange("p (t c) -> p t c", c=CH)
                        zd3 = zd[:].rearrange("p (t c) -> p t c", c=CH)
                        for t0, nt, w in by_sc_s[sc]:
                            rt = t0 - sc * SCT
                            nc.gpsimd.dma_gather(
                                za3[:, rt:rt + nt, :],
                                z3f[w * w0:(w + 1) * w0, :],
                                six[:, rt * 8:rt * 8 + nt * 8],
                                nt * P, nt * P, CH, queue_num=nextq())
                        for t0, nt, w in by_sc_d[sc]:
                            rt = t0 - sc * SCT
                            nc.gpsimd.dma_gather(
                                zd3[:, rt:rt + nt, :],
                                z3f[w * w0:(w + 1) * w0, :],
                                dix[:, rt * 8:rt * 8 + nt * 8],
                                nt * P, nt * P, CH, queue_num=nextq())
                        prod = prdp.tile([P, SCT * CH], bf16, tag="prod")
                        nc.vector.tensor_tensor(
                            out=prod[:, :nt_sc * CH],
                            in0=za[:, :nt_sc * CH], in1=zd[:, :nt_sc * CH],
                            op=mybir.AluOpType.mult)
                        sc_t = zbp.tile([P, SCT], f32, tag="sc")
                        nc.vector.tensor_reduce(
                            out=sc_t[:, :nt_sc],
                            in_=prod[:, :nt_sc * CH]
                                .rearrange("p (t c) -> p t c", c=CH),
                            axis=mybir.AxisListType.X,
                            op=mybir.AluOpType.add)
                        tp = psp.tile([P, P], f32, tag="tp")
                        nc.tensor.transpose(out=tp[:nt_sc, :],
                                            in_=sc_t[:, :nt_sc],
                                            identity=ident[:])
                        so = zbp.tile([SCT, P], f32, tag="so")
                        nc.vector.tensor_copy(out=so[:nt_sc, :],
                                              in_=tp[:nt_sc, :])
                        nc.sync.dma_start(
                            out=out_t[sc * SCT:sc * SCT + nt_sc, :],
                            in_=so[:nt_sc, :])

            def run_pipeline(r):
                hf1, hf2, hf3, z3f = hf1s[r], hf2s[r], hf3s[r], z3fs[r]
                dense(x_s, W1_sb, hs1, "dense1")
                allgather(hs1, hf1, "ag1")
                agg(hf1, bb1_sb, True, z1s, f32, "agg1")
                dense(z1s, W2_sb, hs2, "dense2")
                allgather(hs2, hf2, "ag2")
                agg(hf2, bb2_sb, True, z2s, f32, "agg2")
                dense(z2s, W3_sb, hs3, "dense3")
                allgather(hs3, hf3, "ag3")
                agg(hf3, bb3_sb, False, z3s, bf16, "agg3")
                allgather(z3s, z3f, "ag4")
                decode(dplan_p, z3f, ps_idx, pd_idx, dec_ops[(0, "p")],
                       dec_ops[(1, "p")], pos_out, "dec_pos")
                decode(dplan_n, z3f, ns_idx, nd_idx, dec_ops[(0, "n")],
                       dec_ops[(1, "n")], neg_out, "dec_neg")

            # NOTE: dma_gather misbehaves inside tc.For_i hardware loops
            # (deterministic corruption, see session notes), so reps are
            # Python-unrolled; pools rotate across reps for overlap.
            for r in range(reps):
                run_pipeline(r)
    nc.compile()
    return nc


_CACHE = {}


def _make_in_maps(x, W1, b1, W2, b2, W3, b3, pe, ne):
    aplan, agg_cores = _prep_agg(pe)
    dplan_p, dec_p = _prep_decode(pe)
    dplan_n, dec_n = _prep_decode(ne)
    W3p = np.zeros((CH, CH), np.float32); W3p[:, :COUT] = W3
    b3p = np.zeros(CH, np.float32); b3p[:COUT] = b3
    in_maps = []
    for c in range(M):
        xs = np.zeros((SLICE, CIN), np.float32)
        xs[:NPC] = x[c * NPC:(c + 1) * NPC]
        a = agg_cores[c]
        in_maps.append({
            "x_s": xs, "W1": W1, "W2": W2, "W3": W3p,
            "ident_in": np.eye(P, dtype=np.float32),
            "iota_in": np.tile(np.arange(P, dtype=np.float32)[None, :],
                               (P, 1)),
            "bb1": np.tile(b1[None, :], (P, 1)).astype(np.float32),
            "bb2": np.tile(b2[None, :], (P, 1)).astype(np.float32),
            "bb3": np.tile(b3p[None, :], (P, 1)).astype(np.float32),
            "dinvT": a["dinvT"], "srcidx": a["srcidx"],
            "dstlocT": a["dstlocT"],
            "ps_idx": dec_p[c]["sidx"], "pd_idx": dec_p[c]["didx"],
            "ns_idx": dec_n[c]["sidx"], "nd_idx": dec_n[c]["didx"],
        })
    meta = dict(aplan=aplan, dplan_p=dplan_p, dplan_n=dplan_n,
                emap_p=[dec_p[c]["emap"] for c in range(M)],
                emap_n=[dec_n[c]["emap"] for c in range(M)])
    return meta, in_maps


def _unpermute(flat_scores, emap):
    out = np.zeros(EPC, np.float32)
    valid = emap >= 0
    out[emap[valid]] = flat_scores[valid]
    return out


def _run(meta, in_maps, reps=1):
    key = (meta["aplan"]["T"], meta["dplan_p"]["TDp"],
           meta["dplan_n"]["TDp"], reps)
    if key not in _CACHE:
        _CACHE[key] = build_nc(meta["aplan"], meta["dplan_p"],
                               meta["dplan_n"], reps=reps)
    res = run_bass_kernel_spmd(_CACHE[key], in_maps,
                               core_ids=list(range(M)))
    pos = np.concatenate(
        [_unpermute(res.results[c]["pos_out"].ravel(), meta["emap_p"][c])
         for c in range(M)])
    neg = np.concatenate(
        [_unpermute(res.results[c]["neg_out"].ravel(), meta["emap_n"][c])
         for c in range(M)])
    return pos, neg


def kernel(x, W1, b1, W2, b2, W3, b3, pos_edge_index, neg_edge_index):
    x = np.asarray(x, dtype=np.float32)
    W1 = np.asarray(W1, np.float32); b1 = np.asarray(b1, np.float32)
    W2 = np.asarray(W2, np.float32); b2 = np.asarray(b2, np.float32)
    W3 = np.asarray(W3, np.float32); b3 = np.asarray(b3, np.float32)
    pe = np.asarray(pos_edge_index).astype(np.int64)
    ne = np.asarray(neg_edge_index).astype(np.int64)
    meta, in_maps = _make_in_maps(x, W1, b1, W2, b2, W3, b3, pe, ne)
    return _run(meta, in_maps, reps=1)


# revision 15
# speedup vs baseline: 3.2318x; 3.2318x over previous
"""GCN link predictor on 8 trn2 NeuronCores (Bass/Tile SPMD) — v3.

v1 (13.76 ms) issued ~11.8k per-tile indirect DMAs at ~1.16 us fixed SWDGE
cost each.  v3 replaces every gather with batched dma_gather (custom Q7
ucode): one op moves up to 1024 rows, ops rotate across 4 SWDGE queues so
descriptor generation runs on all 8 Q7 cores.

dma_gather needs int16 indices, so node tables are windowed: 4 windows of
N/4 (<=32767) rows; edge slots are sorted so each gather op touches one
window.  agg slots: [dst-block][window] with per-(b,w) runs padded to 128;
decode slots: 16 (src-window, dst-window) classes, each padded to 128.

bf16 everywhere on the gather path (hhat / AllGather / hf / msg / S / z3);
layer 3 is zero-padded to 128 cols so every gathered row is 256 B.
PSUM accumulation stays fp32.  Scores come back per-slot; the host
un-permutes slots back to edge order.

Self-contained: hardcodes all shapes for the nn_GCNLinkPredictor problem.
"""
import numpy as np

import concourse.bacc as bacc
import concourse.bass as bass
import concourse.mybir as mybir
import concourse.tile as tile
from concourse.bass_utils import run_bass_kernel_spmd

P = 128
N = 100000
E = 1600000
M = 8
NW = 4                       # index windows (N/NW must be <= 32767)
NPC = N // M                 # 12500
BPC = (NPC + P - 1) // P     # 98
SLICE = BPC * P              # 12544
CIN = 128
CH = 128                     # layer width; layer 3 zero-padded to CH
COUT = 64
EPC = E // M                 # 200000
SCT = 32                     # decode superchunk tiles
OPT = 8                      # max tiles per dma_gather op (1024 idxs)
NSWQ = 4                     # SWDGE queues (desc-gen parallelism)

f32 = mybir.dt.float32
bf16 = mybir.dt.bfloat16
i16 = mybir.dt.int16
i32 = mybir.dt.int32


def _configure(n, e):
    global N, E, NPC, BPC, SLICE, EPC
    N, E = n, e
    NPC = N // M
    BPC = (NPC + P - 1) // P
    SLICE = BPC * P
    EPC = E // M


def _w0():
    w0 = N // NW
    assert N % NW == 0 and w0 <= 32767
    return w0


def _pack_idx16(flat):
    """flat slot-ordered local indices -> [128, len/16] int16 (16-part
    wrap, replicated for the 8 Q7 cores)."""
    assert len(flat) % 16 == 0
    a = flat.reshape(-1, 16).T.astype(np.int16)
    return np.ascontiguousarray(np.tile(a, (8, 1)))


# --------------------------- host preprocessing ---------------------------

def _prep_agg(pos_edge_index):
    """Slot layout: for b in blocks: for w in windows: K[b][w] tiles."""
    w0 = _w0()
    src = np.concatenate([pos_edge_index[0], np.arange(N, dtype=np.int64)])
    dst = np.concatenate([pos_edge_index[1], np.arange(N, dtype=np.int64)])
    deg = np.bincount(dst, minlength=N).astype(np.float32)
    dinv = np.where(deg > 0, 1.0 / np.sqrt(deg), 0.0).astype(np.float32)

    core_of = dst // NPC
    per_core = []
    counts = np.zeros((M, BPC, NW), dtype=np.int64)
    for c in range(M):
        sel = core_of == c
        s_c, d_c = src[sel], dst[sel]
        b_c = (d_c - c * NPC) // P
        w_c = s_c // w0
        order = np.lexsort((w_c, b_c))
        s_c, d_c, b_c, w_c = s_c[order], d_c[order], b_c[order], w_c[order]
        np.add.at(counts[c], (b_c, w_c), 1)
        per_core.append((s_c, d_c, b_c, w_c))

    K = np.max((counts + P - 1) // P, axis=0)        # [BPC, NW] tiles
    KB = K.sum(axis=1)                               # tiles per block
    T = int(KB.sum())

    cores = []
    for c in range(M):
        s_c, d_c, b_c, w_c = per_core[c]
        srcloc = np.zeros(T * P, dtype=np.int16)
        dstloc = np.full(T * P, -1.0, dtype=np.float32)
        bw = b_c * NW + w_c
        starts = np.searchsorted(bw, np.arange(BPC * NW))
        ends = np.searchsorted(bw, np.arange(BPC * NW) + 1)
        tile_off = 0
        for b in range(BPC):
            for w in range(NW):
                n_bw = ends[b * NW + w] - starts[b * NW + w]
                base = tile_off * P
                sl = slice(starts[b * NW + w], ends[b * NW + w])
                srcloc[base:base + n_bw] = (s_c[sl] - w * w0).astype(np.int16)
                dstloc[base:base + n_bw] = (
                    d_c[sl] - (c * NPC + b * P)).astype(np.float32)
                tile_off += K[b, w]
        dinvT = np.zeros((P, BPC), dtype=np.float32)
        for b in range(BPC):
            lo = c * NPC + b * P
            hi = min(lo + P, (c + 1) * NPC)
            dinvT[:hi - lo, b] = dinv[lo:hi]
        cores.append(dict(
            srcidx=_pack_idx16(srcloc),
            dstlocT=np.ascontiguousarray(dstloc.reshape(T, P).T),
            dinvT=dinvT))
    return dict(K=K, KB=KB, T=T), cores


def _prep_decode(edge_index):
    """Sort each core's edges into 16 (ws, wd) classes, pad to 128-mult."""
    w0 = _w0()
    per_core = []
    counts = np.zeros((M, NW * NW), dtype=np.int64)
    for c in range(M):
        s = edge_index[0, c * EPC:(c + 1) * EPC].astype(np.int64)
        d = edge_index[1, c * EPC:(c + 1) * EPC].astype(np.int64)
        cls = (s // w0) * NW + (d // w0)
        order = np.argsort(cls, kind="stable")
        s, d, cls = s[order], d[order], cls[order]
        np.add.at(counts[c], cls, 1)
        per_core.append((s, d, cls, order))

    KD = np.max((counts + P - 1) // P, axis=0)       # [16] tiles per class
    TDp = int(KD.sum())

    cores = []
    for c in range(M):
        s, d, cls, order = per_core[c]
        sidx = np.zeros(TDp * P, dtype=np.int16)
        didx = np.zeros(TDp * P, dtype=np.int16)
        emap = np.full(TDp * P, -1, dtype=np.int64)
        starts = np.searchsorted(cls, np.arange(NW * NW))
        ends = np.searchsorted(cls, np.arange(NW * NW) + 1)
        toff = 0
        for k in range(NW * NW):
            ws, wd = k // NW, k % NW
            n_k = ends[k] - starts[k]
            base = toff * P
            sl = slice(starts[k], ends[k])
            sidx[base:base + n_k] = (s[sl] - ws * w0).astype(np.int16)
            didx[base:base + n_k] = (d[sl] - wd * w0).astype(np.int16)
            emap[base:base + n_k] = order[sl]
            toff += KD[k]
        cores.append(dict(sidx=_pack_idx16(sidx), didx=_pack_idx16(didx),
                          emap=emap))
    return dict(KD=KD, TDp=TDp), cores


def _gather_ops_agg(plan):
    K = plan["K"]
    ops = []
    toff = 0
    for b in range(BPC):
        for w in range(NW):
            k = int(K[b, w])
            o = 0
            while o < k:
                n = min(OPT, k - o)
                ops.append((toff + o, n, w))
                o += n
            toff += k
    return ops


def _gather_ops_decode(plan, side):
    KD = plan["KD"]
    ops = []
    toff = 0
    for k in range(NW * NW):
        w = (k // NW) if side == 0 else (k % NW)
        kt = int(KD[k])
        o = 0
        while o < kt:
            t = toff + o
            room_sc = SCT - (t % SCT)
            n = min(OPT, kt - o, room_sc)
            ops.append((t, n, w))
            o += n
        toff += kt
    return ops


# ----------------------------- device builder -----------------------------

def build_nc(aplan, dplan_p, dplan_n, reps=1):
    T = aplan["T"]
    K = aplan["K"]
    TDp, TDn = dplan_p["TDp"], dplan_n["TDp"]
    w0 = _w0()

    nc = bacc.Bacc(None, target_bir_lowering=False, num_swdge_queues=NSWQ)
    qrot = [0]

    def nextq():
        q = qrot[0]
        qrot[0] = (q + 1) % NSWQ
        return q

    with tile.TileContext(nc) as tc:
        with tc.tile_pool(name="dram", bufs=1, space="DRAM") as dram, \
             tc.tile_pool(name="cst", bufs=1) as cst, \
             tc.tile_pool(name="xt", bufs=3) as xtp, \
             tc.tile_pool(name="sS", bufs=3) as sSp, \
             tc.tile_pool(name="msg", bufs=3) as msgp, \
             tc.tile_pool(name="zb", bufs=4) as zbp, \
             tc.tile_pool(name="dg", bufs=2) as dgp, \
             tc.tile_pool(name="prd", bufs=2) as prdp, \
             tc.tile_pool(name="dix", bufs=3) as dixp, \
             tc.tile_pool(name="ps", bufs=2, space="PSUM") as psp, \
             tc.tile_pool(name="acc", bufs=2, space="PSUM") as accp:

            def ein(name, shape, dtype=f32):
                return dram.tile(shape, dtype, kind="ExternalInput",
                                 name=name, uniquify=False)

            x_s = ein("x_s", [SLICE, CIN])
            ident_in = ein("ident_in", [P, P])
            iota_in = ein("iota_in", [P, P])
            W1 = ein("W1", [CIN, CH]); W2 = ein("W2", [CH, CH])
            W3 = ein("W3", [CH, CH])                      # zero-padded
            bb1 = ein("bb1", [P, CH]); bb2 = ein("bb2", [P, CH])
            bb3 = ein("bb3", [P, CH])                     # zero-padded
            dinvT = ein("dinvT", [P, BPC])
            srcidx = ein("srcidx", [P, T * 8], i16)
            dstlocT = ein("dstlocT", [P, T])
            ps_idx = ein("ps_idx", [P, TDp * 8], i16)
            pd_idx = ein("pd_idx", [P, TDp * 8], i16)
            ns_idx = ein("ns_idx", [P, TDn * 8], i16)
            nd_idx = ein("nd_idx", [P, TDn * 8], i16)

            pos_out = dram.tile([TDp, P], f32, kind="ExternalOutput",
                                name="pos_out", uniquify=False)
            neg_out = dram.tile([TDn, P], f32, kind="ExternalOutput",
                                name="neg_out", uniquify=False)

            hs1 = dram.tile([SLICE, CH], bf16, name="hs1")
            hs2 = dram.tile([SLICE, CH], bf16, name="hs2")
            hs3 = dram.tile([SLICE, CH], bf16, name="hs3")
            z1s = dram.tile([SLICE, CH], f32, name="z1s")
            z2s = dram.tile([SLICE, CH], f32, name="z2s")
            z3s = dram.tile([SLICE, CH], bf16, name="z3s")
            hf1 = dram.tile([N, CH], bf16, name="hf1", addr_space="Shared")
            hf2 = dram.tile([N, CH], bf16, name="hf2", addr_space="Shared")
            hf3 = dram.tile([N, CH], bf16, name="hf3", addr_space="Shared")
            z3f = dram.tile([N, CH], bf16, name="z3f", addr_space="Shared")

            # ---------------- constants to SBUF ----------------
            W1_sb = cst.tile([CIN, CH], f32)
            W2_sb = cst.tile([CH, CH], f32)
            W3_sb = cst.tile([CH, CH], f32)
            bb1_sb = cst.tile([P, CH], f32)
            bb2_sb = cst.tile([P, CH], f32)
            bb3_sb = cst.tile([P, CH], f32)
            dinv_sb = cst.tile([P, BPC], f32)
            srcidx_sb = cst.tile([P, T * 8], i16)
            dstloc_sb = cst.tile([P, T], f32)
            ident = cst.tile([P, P], f32)
            iota_f = cst.tile([P, P], f32)
            for dst_t, src_t in [(W1_sb, W1), (W2_sb, W2), (W3_sb, W3),
                                 (bb1_sb, bb1), (bb2_sb, bb2), (bb3_sb, bb3),
                                 (dinv_sb, dinvT), (srcidx_sb, srcidx),
                                 (dstloc_sb, dstlocT), (ident, ident_in),
                                 (iota_f, iota_in)]:
                nc.sync.dma_start(out=dst_t[:], in_=src_t[:])

            agg_ops = _gather_ops_agg(aplan)
            dec_ops = {(0, "p"): _gather_ops_decode(dplan_p, 0),
                       (1, "p"): _gather_ops_decode(dplan_p, 1),
                       (0, "n"): _gather_ops_decode(dplan_n, 0),
                       (1, "n"): _gather_ops_decode(dplan_n, 1)}

            # ---------------- phases ----------------
            def dense(z_in, W_sb, hs_out, scope):
                with nc.named_scope(scope):
                    for i in range(BPC):
                        zt = xtp.tile([P, CH], f32, tag="zt")
                        nc.sync.dma_start(out=zt[:],
                                          in_=z_in[i * P:(i + 1) * P, :])
                        tp = psp.tile([P, CH], f32, tag="tp")
                        nc.tensor.transpose(out=tp[:], in_=zt[:],
                                            identity=ident[:])
                        zT = xtp.tile([P, CH], f32, tag="zT")
                        nc.vector.tensor_copy(out=zT[:], in_=tp[:])
                        hp = psp.tile([P, CH], f32, tag="hp")
                        nc.tensor.matmul(out=hp[:], lhsT=zT[:], rhs=W_sb[:],
                                         start=True, stop=True)
                        hh = zbp.tile([P, CH], bf16, tag="hh")
                        nc.vector.tensor_scalar(
                            out=hh[:], in0=hp[:],
                            scalar1=dinv_sb[:, i:i + 1], scalar2=None,
                            op0=mybir.AluOpType.mult)
                        nc.sync.dma_start(
                            out=hs_out[i * P:(i + 1) * P, :], in_=hh[:])

            def allgather(slice_t, full_t, scope):
                with nc.named_scope(scope):
                    nc.gpsimd.collective_compute(
                        "AllGather", mybir.AluOpType.bypass,
                        replica_groups=[list(range(M))],
                        ins=[slice_t[:NPC, :]],
                        outs=[full_t[:]])

            def agg(hf, bias_sb, relu, z_out, zdt, scope):
                with nc.named_scope(scope):
                    opi = 0
                    toff = 0
                    for b in range(BPC):
                        kb = int(K[b].sum())
                        msgw = msgp.tile([P, kb * CH], bf16, tag="msg")
                        m3 = msgw[:].rearrange("p (t c) -> p t c", c=CH)
                        while opi < len(agg_ops) and \
                                agg_ops[opi][0] < toff + kb:
                            t0, nt, w = agg_ops[opi]
                            nc.gpsimd.dma_gather(
                                m3[:, t0 - toff:t0 - toff + nt, :],
                                hf[w * w0:(w + 1) * w0, :],
                                srcidx_sb[:, t0 * 8:t0 * 8 + nt * 8],
                                nt * P, nt * P, CH, queue_num=nextq())
                            opi += 1
                        S = sSp.tile([P, kb * P], bf16, tag="S")
                        nc.vector.tensor_tensor(
                            out=S[:].rearrange("p (k q) -> p k q", q=P),
                            in0=dstloc_sb[:, toff:toff + kb][:, :, None]
                                .broadcast_to([P, kb, P]),
                            in1=iota_f[:][:, None, :]
                                .broadcast_to([P, kb, P]),
                            op=mybir.AluOpType.is_equal)
                        acc = accp.tile([P, CH], f32, tag="acc")
                        for k in range(kb):
                            nc.tensor.matmul(
                                out=acc[:],
                                lhsT=S[:, k * P:(k + 1) * P],
                                rhs=msgw[:, k * CH:(k + 1) * CH],
                                start=(k == 0), stop=(k == kb - 1))
                        zb = zbp.tile([P, CH], f32, tag="zb")
                        nc.vector.tensor_scalar(
                            out=zb[:], in0=acc[:],
                            scalar1=dinv_sb[:, b:b + 1], scalar2=None,
                            op0=mybir.AluOpType.mult)
                        if relu:
                            nc.vector.tensor_tensor(
                                out=zb[:], in0=zb[:], in1=bias_sb[:],
                                op=mybir.AluOpType.add)
                            zo = zbp.tile([P, CH], zdt, tag="zo")
                            nc.vector.tensor_scalar_max(zo[:], zb[:], 0.0)
                        else:
                            zo = zbp.tile([P, CH], zdt, tag="zo")
                            nc.vector.tensor_tensor(
                                out=zo[:], in0=zb[:], in1=bias_sb[:],
                                op=mybir.AluOpType.add)
                        nc.sync.dma_start(
                            out=z_out[b * P:(b + 1) * P, :], in_=zo[:])
                        toff += kb

            def decode(plan, z3f_t, sidx_t, didx_t, ops_s, ops_d, out_t,
                       scope):
                TDx = plan["TDp"]
                nsc = (TDx + SCT - 1) // SCT
                by_sc_s = [[] for _ in range(nsc)]
                for op in ops_s:
                    by_sc_s[op[0] // SCT].append(op)
                by_sc_d = [[] for _ in range(nsc)]
                for op in ops_d:
                    by_sc_d[op[0] // SCT].append(op)
                with nc.named_scope(scope):
                    for sc in range(nsc):
                        nt_sc = min(SCT, TDx - sc * SCT)
                        six = dixp.tile([P, SCT * 8], i16, tag="six")
                        nc.sync.dma_start(
                            out=six[:, :nt_sc * 8],
                            in_=sidx_t[:, sc * SCT * 8:
                                       sc * SCT * 8 + nt_sc * 8])
                        dix = dixp.tile([P, SCT * 8], i16, tag="dix")
                        nc.sync.dma_start(
                            out=dix[:, :nt_sc * 8],
                            in_=didx_t[:, sc * SCT * 8:
                                       sc * SCT * 8 + nt_sc * 8])
                        za = dgp.tile([P, SCT * CH], bf16, tag="za")
                        zd = dgp.tile([P, SCT * CH], bf16, tag="zd")
                        za3 = za[:].rearrange("p (t c) -> p t c", c=CH)
                        zd3 = zd[:].rearrange("p (t c) -> p t c", c=CH)
                        for t0, nt, w in by_sc_s[sc]:
                            rt = t0 - sc * SCT
                            nc.gpsimd.dma_gather(
                                za3[:, rt:rt + nt, :],
                                z3f_t[w * w0:(w + 1) * w0, :],
                                six[:, rt * 8:rt * 8 + nt * 8],
                                nt * P, nt * P, CH, queue_num=nextq())
                        for t0, nt, w in by_sc_d[sc]:
                            rt = t0 - sc * SCT
                            nc.gpsimd.dma_gather(
                                zd3[:, rt:rt + nt, :],
                                z3f_t[w * w0:(w + 1) * w0, :],
                                dix[:, rt * 8:rt * 8 + nt * 8],
                                nt * P, nt * P, CH, queue_num=nextq())
                        prod = prdp.tile([P, SCT * CH], bf16, tag="prod")
                        nc.vector.tensor_tensor(
                            out=prod[:, :nt_sc * CH],
                            in0=za[:, :nt_sc * CH], in1=zd[:, :nt_sc * CH],
                            op=mybir.AluOpType.mult)
                        sc_t = zbp.tile([P, SCT], f32, tag="sc")
                        nc.vector.tensor_reduce(
                            out=sc_t[:, :nt_sc],
                            in_=prod[:, :nt_sc * CH]
                                .rearrange("p (t c) -> p t c", c=CH),
                            axis=mybir.AxisListType.X,
                            op=mybir.AluOpType.add)
                        tp = psp.tile([P, P], f32, tag="tp")
                        nc.tensor.transpose(out=tp[:nt_sc, :],
                                            in_=sc_t[:, :nt_sc],
                                            identity=ident[:])
                        so = zbp.tile([SCT, P], f32, tag="so")
                        nc.vector.tensor_copy(out=so[:nt_sc, :],
                                              in_=tp[:nt_sc, :])
                        nc.sync.dma_start(
                            out=out_t[sc * SCT:sc * SCT + nt_sc, :],
                            in_=so[:nt_sc, :])

            def run_pipeline():
                dense(x_s, W1_sb, hs1, "dense1")
                allgather(hs1, hf1, "ag1")
                agg(hf1, bb1_sb, True, z1s, f32, "agg1")
                dense(z1s, W2_sb, hs2, "dense2")
                allgather(hs2, hf2, "ag2")
                agg(hf2, bb2_sb, True, z2s, f32, "agg2")
                dense(z2s, W3_sb, hs3, "dense3")
                allgather(hs3, hf3, "ag3")
                agg(hf3, bb3_sb, False, z3s, bf16, "agg3")
                allgather(z3s, z3f, "ag4")
                decode(dplan_p, z3f, ps_idx, pd_idx, dec_ops[(0, "p")],
                       dec_ops[(1, "p")], pos_out, "dec_pos")
                decode(dplan_n, z3f, ns_idx, nd_idx, dec_ops[(0, "n")],
                       dec_ops[(1, "n")], neg_out, "dec_neg")

            if reps > 1:
                with tc.For_i(0, reps, 1):
                    run_pipeline()
            else:
                run_pipeline()
    nc.compile()
    return nc


_CACHE = {}


def _make_in_maps(x, W1, b1, W2, b2, W3, b3, pe, ne):
    aplan, agg_cores = _prep_agg(pe)
    dplan_p, dec_p = _prep_decode(pe)
    dplan_n, dec_n = _prep_decode(ne)
    W3p = np.zeros((CH, CH), np.float32); W3p[:, :COUT] = W3
    b3p = np.zeros(CH, np.float32); b3p[:COUT] = b3
    in_maps = []
    for c in range(M):
        xs = np.zeros((SLICE, CIN), np.float32)
        xs[:NPC] = x[c * NPC:(c + 1) * NPC]
        a = agg_cores[c]
        in_maps.append({
            "x_s": xs, "W1": W1, "W2": W2, "W3": W3p,
            "ident_in": np.eye(P, dtype=np.float32),
            "iota_in": np.tile(np.arange(P, dtype=np.float32)[None, :],
                               (P, 1)),
            "bb1": np.tile(b1[None, :], (P, 1)).astype(np.float32),
            "bb2": np.tile(b2[None, :], (P, 1)).astype(np.float32),
            "bb3": np.tile(b3p[None, :], (P, 1)).astype(np.float32),
            "dinvT": a["dinvT"], "srcidx": a["srcidx"],
            "dstlocT": a["dstlocT"],
            "ps_idx": dec_p[c]["sidx"], "pd_idx": dec_p[c]["didx"],
            "ns_idx": dec_n[c]["sidx"], "nd_idx": dec_n[c]["didx"],
        })
    meta = dict(aplan=aplan, dplan_p=dplan_p, dplan_n=dplan_n,
                emap_p=[dec_p[c]["emap"] for c in range(M)],
                emap_n=[dec_n[c]["emap"] for c in range(M)])
    return meta, in_maps


def _unpermute(flat_scores, emap):
    out = np.zeros(EPC, np.float32)
    valid = emap >= 0
    out[emap[valid]] = flat_scores[valid]
    return out


def _run(meta, in_maps, reps=1):
    key = (meta["aplan"]["T"], meta["dplan_p"]["TDp"],
           meta["dplan_n"]["TDp"], reps)
    if key not in _CACHE:
        _CACHE[key] = build_nc(meta["aplan"], meta["dplan_p"],
                               meta["dplan_n"], reps=reps)
    res = run_bass_kernel_spmd(_CACHE[key], in_maps,
                               core_ids=list(range(M)))
    pos = np.concatenate(
        [_unpermute(res.results[c]["pos_out"].ravel(), meta["emap_p"][c])
         for c in range(M)])
    neg = np.concatenate(
        [_unpermute(res.results[c]["neg_out"].ravel(), meta["emap_n"][c])
         for c in range(M)])
    return pos, neg


def kernel(x, W1, b1, W2, b2, W3, b3, pos_edge_index, neg_edge_index):
    x = np.asarray(x, dtype=np.float32)
    W1 = np.asarray(W1, np.float32); b1 = np.asarray(b1, np.float32)
    W2 = np.asarray(W2, np.float32); b2 = np.asarray(b2, np.float32)
    W3 = np.asarray(W3, np.float32); b3 = np.asarray(b3, np.float32)
    pe = np.asarray(pos_edge_index).astype(np.int64)
    ne = np.asarray(neg_edge_index).astype(np.int64)
    meta, in_maps = _make_in_maps(x, W1, b1, W2, b2, W3, b3, pe, ne)
    return _run(meta, in_maps, reps=1)
